# revision 12
# baseline (speedup 1.0000x reference)
"""Trainium2 Bass kernel for low-rank-QK multi-head attention.

Reference computation (B=4, S=2048, HIDDEN=2048, HEADS=16, R=128):
    Q = q @ wqs_w + wqs_b                    # [B, S, 16]
    K = k @ wks_w + wks_b                    # [B, S, 16]
    V = k @ wvs_w + wvs_b                    # [B, S, 2048]   (v input unused)
    logits = Q @ K^T / sqrt(128)             # [B, S, S]
    score = softmax(logits, -1)
    out = (score @ V) @ wo_w + wo_b          # [B, S, 2048]

Sharding: 8 cores = (batch b, query-half h).  Each core handles the full
key set of its batch and a 1024-row query slice.

Algebraic restructuring (host precompute on weights only):
  score @ (k @ wvs) @ wo  =  (score @ k) @ (wvs @ wo)  =  (score @ k) @ W2
W2 = wvs_w @ wo_w is computed on host once per call (weights are runtime
inputs), removing one of the three S*H*H matmuls entirely.  Softmax
normalization is deferred: ET = exp(logits) @ k, out = (ET @ W2) * 1/Z.
Bias fold: softmax rows sum to 1, so wvs_b/wo_b contribute the constant
row wvs_b @ wo_w + wo_b, added on host.

Device-side dataflow (contraction dims on SBUF partitions, no on-device
transposes; matmul operands bf16, PSUM fp32):
  - host supplies qT = q^T (shard), kT = k^T (batch), kn = k in
    [128, tile, hid] layout, W2 in [128, nblk, chunk, 512] layout.
  - QT[h, sq]  = sum_c wqs_c^T qT_c          (16-row result, zero-padded)
  - KT[h, sk]  = sum_c wks_c^T kT_c
  - logitsT_j[k, q] = KT_j^T QT              (keys on partitions)
  - expT_j = exp(logitsT_j / sqrt(R))        (ScalarE, full 1024-wide)
  - Z accumulated j-outer right behind each exp tile (8 psum columns)
  - ET[hid, q] = sum_j kn_j^T expT_j         (= (exp @ k)^T)
  - out[q, n]  = (sum_c ET_c^T W2_c) * (1/Z[q])   per-partition scale
"""

import math
import sys

import numpy as np

if "/opt/trn_rl_repo" not in sys.path:
    sys.path.insert(0, "/opt/trn_rl_repo")

import ml_dtypes

BF = ml_dtypes.bfloat16

HIDDEN = 2048
HEADS = 16
R = 128
B = 4
S = 2048

P = 128
SQ = 1024  # queries per core
SK = 2048  # keys per core (full batch)
HC = HIDDEN // P  # 16 hidden-dim chunks
NKT = SK // P  # 16 key tiles
NQT = SQ // P  # 8 query tiles
N512_Q = SQ // 512  # 2
N512_K = SK // 512  # 4
N512_D = HIDDEN // 512  # 4
ISQRT_R = 1.0 / math.sqrt(R)

# Module-level knobs for test harness (harness itself only calls kernel()).
TRACE = False
TRACE_KWARGS = {}
LAST_RESULTS = None

_PROG = None


def _emit(tc, nc, mybir, ap):
    """Emit the single-core SPMD program body."""
    from contextlib import ExitStack

    f32 = mybir.dt.float32
    bf16 = mybir.dt.bfloat16
    Exp = mybir.ActivationFunctionType.Exp

    with ExitStack() as ctx:
        # ---- long-lived small tiles -------------------------------------
        small = ctx.enter_context(tc.tile_pool(name="small", bufs=1))
        QTs = small.tile([P, SQ], bf16, name="QTs")  # zero-padded 16->128
        KTs = small.tile([P, SK], bf16, name="KTs")
        ones = small.tile([P, 1], bf16, name="ones")
        recip = small.tile([P, NQT], f32, name="recip")
        qb = small.tile([HEADS, 1], f32, name="qb")
        kb = small.tile([HEADS, 1], f32, name="kb")
        wqs_sb = small.tile([P, HC, HEADS], bf16, name="wqs_sb")
        wks_sb = small.tile([P, HC, HEADS], bf16, name="wks_sb")

        # W2 stream pool lives below kn/expT on the stack so its DMAs don't
        # serialize behind the ET phase's last reads of that space (and so
        # the later kn/expT releases keep LIFO order).
        w2s = ctx.enter_context(tc.tile_pool(name="w2s", bufs=2))
        w2_tiles = []

        expT_pool = tc.alloc_tile_pool(name="expT", bufs=1)
        expT = expT_pool.tile([P, NKT, SQ], bf16, name="expT")
        kn_pool = tc.alloc_tile_pool(name="knp", bufs=1)
        kn_sb = kn_pool.tile([P, NKT, HIDDEN], bf16, name="kn_sb")

        def load_w2(n, eng):
            t = w2s.tile([P, HC, 512], bf16, name="w2_t", tag="w2")
            eng.dma_start(t[:], ap["w2"][:, n, :, :])
            w2_tiles.append(t)


        nc.vector.memset(ones[:], 1.0)
        nc.vector.memset(QTs[:], 0.0)
        nc.vector.memset(KTs[:], 0.0)
        nc.sync.dma_start(qb[:], ap["wqs_b"][:])
        nc.sync.dma_start(kb[:], ap["wks_b"][:])
        nc.sync.dma_start(wqs_sb[:], ap["wqs"].rearrange("(c p) h -> p c h", p=P))
        nc.sync.dma_start(wks_sb[:], ap["wks"].rearrange("(c p) h -> p c h", p=P))
        # (kn chunk loads are interleaved into the projection loop below so
        # the latency-critical kT/qT streams win the DMA queues at start.)

        # ================= phase 1: Q/K projections ======================
        with tc.tile_pool(name="kts", bufs=2) as kts, \
             tc.tile_pool(name="qts", bufs=2) as qts, \
             tc.tile_pool(name="ps_kt", bufs=1, space="PSUM") as ps_ktp, \
             tc.tile_pool(name="ps_qt", bufs=1, space="PSUM") as ps_qtp:
            ps_kt = ps_ktp.tile([HEADS, SK], f32, name="ps_kt")
            ps_qt = ps_qtp.tile([HEADS, SQ], f32, name="ps_qt")
            for c in range(HC):
                kt_c = kts.tile([P, SK], bf16, name="kt_c", tag="kt")
                nc.sync.dma_start(kt_c[:], ap["kT"][c * P:(c + 1) * P, :])
                for n in range(N512_K):
                    nc.tensor.matmul(
                        ps_kt[:, n * 512:(n + 1) * 512], wks_sb[:, c, :],
                        kt_c[:, n * 512:(n + 1) * 512],
                        start=(c == 0), stop=(c == HC - 1),
                    )
                qt_c = qts.tile([P, SQ], bf16, name="qt_c", tag="qt")
                nc.sync.dma_start(qt_c[:], ap["qT"][c * P:(c + 1) * P, :])
                for n in range(N512_Q):
                    nc.tensor.matmul(
                        ps_qt[:, n * 512:(n + 1) * 512], wqs_sb[:, c, :],
                        qt_c[:, n * 512:(n + 1) * 512],
                        start=(c == 0), stop=(c == HC - 1),
                    )
            nc.vector.tensor_scalar_add(KTs[0:HEADS, :], ps_kt[:], kb[:])
            nc.vector.tensor_scalar_add(QTs[0:HEADS, :], ps_qt[:], qb[:])

        # ET lives on the right-side stack: its lifetime overlaps (not
        # nests with) the expT/kn pools released before the out phase.
        ET_pool = tc.alloc_tile_pool(name="ETp", bufs=1, side="right")
        ET = ET_pool.tile([P, HC, SQ], bf16, name="ET")

        # ============ phase 2: logitsT + exp; phase 3: Z =================
        with tc.tile_pool(name="ps_l", bufs=2, space="PSUM") as ps_l, \
             tc.tile_pool(name="ps_z", bufs=1, space="PSUM") as ps_zp:
            pz = ps_zp.tile([P, NQT], f32, name="pz")
            for j in range(NKT):
                pl = ps_l.tile([P, SQ], f32, name="ps_l", tag="lT")
                for n in range(N512_Q):
                    nc.tensor.matmul(
                        pl[:, n * 512:(n + 1) * 512],
                        KTs[:, j * P:(j + 1) * P],
                        QTs[:, n * 512:(n + 1) * 512],
                        start=True, stop=True,
                    )
                nc.scalar.activation(expT[:, j, :], pl[:], Exp, scale=ISQRT_R)
                # kn/W2 loads issue from the Activation queue, which only
                # reaches them after proj completes (the SP queues drain the
                # latency-critical kT/qT stream unhindered; SP queue FIFOs
                # drain independently, so SP emission order cannot sequence
                # the wire).
                if j < 4:
                    nc.scalar.dma_start(
                        kn_sb[:, 4 * j:4 * (j + 1), :],
                        ap["kn"][:, 4 * j:4 * (j + 1), :],
                    )
            load_w2(0, nc.scalar)
            load_w2(1, nc.scalar)

            # Z trails each exp tile (j-outer) so the PE absorbs the
            # ScalarE drain instead of idling before phase 4.  All eight
            # pz columns share one PSUM bank whose zero-region is marked
            # by a single start; the other columns' first writes land on
            # pending-zero bytes and write through (correct under both
            # lazy and eager bank-zeroing).
            for j in range(NKT):
                for i in range(NQT):
                    nc.tensor.matmul(
                        pz[:, i:i + 1], expT[:, j, i * P:(i + 1) * P], ones[:],
                        start=(j == 0 and i == 0), stop=(j == NKT - 1),
                        skip_group_check=True,
                    )
            nc.vector.reciprocal(recip[:], pz[:])

            # ============= phase 4: ET[hid, q] = sum_j kn_j^T expT_j =====
            with tc.tile_pool(name="ps_e", bufs=2, space="PSUM") as ps_e:
                for ht in range(HC):
                    for n in range(N512_Q):
                        pe = ps_e.tile([P, 512], f32, name="ps_e", tag="e")
                        for j in range(NKT):
                            nc.tensor.matmul(
                                pe[:],
                                kn_sb[:, j, ht * P:(ht + 1) * P],
                                expT[:, j, n * 512:(n + 1) * 512],
                                start=(j == 0), stop=(j == NKT - 1),
                            )
                        nc.vector.tensor_copy(
                            ET[:, ht, n * 512:(n + 1) * 512], pe[:]
                        )

        kn_pool.release()
        expT_pool.release()

        # ================= phase 5: out = (ET^T @ W2) * recip ============
        with tc.tile_pool(name="outs", bufs=3) as outs, \
             tc.tile_pool(name="ps_o", bufs=2, space="PSUM") as ps_o:
            for n in range(N512_D):
                w2_n = w2_tiles[n]
                for i in range(NQT):
                    po = ps_o.tile([P, 512], f32, name="ps_o", tag="o")
                    for c in range(HC):
                        nc.tensor.matmul(
                            po[:], ET[:, c, i * P:(i + 1) * P], w2_n[:, c, :],
                            start=(c == 0), stop=(c == HC - 1),
                        )
                    ot = outs.tile([P, 512], bf16, name="ot", tag="ot")
                    nc.vector.tensor_scalar_mul(ot[:], po[:], recip[:, i:i + 1])
                    nc.sync.dma_start(
                        ap["out"][i * P:(i + 1) * P, n * 512:(n + 1) * 512], ot[:]
                    )
                if n + 2 < N512_D:
                    load_w2(n + 2, nc.sync)
        ET_pool.release()


def _build_program():
    import concourse.tile as tile
    from concourse import bacc, mybir

    f32 = mybir.dt.float32
    bf16 = mybir.dt.bfloat16

    nc = bacc.Bacc(
        "TRN2", debug=False, num_devices=8, dynamic_dma_scratch_size=512
    )

    ap = {
        "qT": nc.dram_tensor("qT", (HIDDEN, SQ), bf16, kind="ExternalInput").ap(),
        "kT": nc.dram_tensor("kT", (HIDDEN, SK), bf16, kind="ExternalInput").ap(),
        "kn": nc.dram_tensor("kn", (P, NKT, HIDDEN), bf16, kind="ExternalInput").ap(),
        "wqs": nc.dram_tensor("wqs", (HIDDEN, HEADS), bf16, kind="ExternalInput").ap(),
        "wks": nc.dram_tensor("wks", (HIDDEN, HEADS), bf16, kind="ExternalInput").ap(),
        "wqs_b": nc.dram_tensor("wqs_b", (HEADS, 1), f32, kind="ExternalInput").ap(),
        "wks_b": nc.dram_tensor("wks_b", (HEADS, 1), f32, kind="ExternalInput").ap(),
        "w2": nc.dram_tensor(
            "w2", (P, N512_D, HC, 512), bf16, kind="ExternalInput"
        ).ap(),
        "out": nc.dram_tensor("out", (SQ, HIDDEN), bf16, kind="ExternalOutput").ap(),
    }

    with tile.TileContext(nc) as tc:
        _emit(tc, nc, mybir, ap)

    nc.compile()
    return nc


def _get_program():
    global _PROG
    if _PROG is None:
        _PROG = _build_program()
    return _PROG


def kernel(q, k, v, wqs_w, wqs_b, wks_w, wks_b, wvs_w, wvs_b, wo_w, wo_b):
    global LAST_RESULTS
    from concourse.bass_utils import run_bass_kernel_spmd

    nc = _get_program()

    q = np.asarray(q, dtype=np.float32)
    k = np.asarray(k, dtype=np.float32)
    wqs_w = np.asarray(wqs_w, dtype=np.float32)
    wqs_b = np.asarray(wqs_b, dtype=np.float32)
    wks_w = np.asarray(wks_w, dtype=np.float32)
    wks_b = np.asarray(wks_b, dtype=np.float32)
    wvs_w = np.asarray(wvs_w, dtype=np.float32)
    wvs_b = np.asarray(wvs_b, dtype=np.float32)
    wo_w = np.asarray(wo_w, dtype=np.float32)
    wo_b = np.asarray(wo_b, dtype=np.float32)

    qbf = q.astype(BF)
    kbf = k.astype(BF)
    wqs = np.ascontiguousarray(wqs_w.astype(BF))
    wks = np.ascontiguousarray(wks_w.astype(BF))
    qb = np.ascontiguousarray(wqs_b.reshape(HEADS, 1))
    kb = np.ascontiguousarray(wks_b.reshape(HEADS, 1))

    # Weight-only precompute: W2 = wvs @ wo, in the [p, nblk, chunk, col]
    # layout the out-phase DMA wants (one contiguous 16KB/partition block
    # per 512-wide output column group).
    W2 = wvs_w @ wo_w
    w2r = np.ascontiguousarray(
        W2.astype(BF).reshape(HC, P, N512_D, 512).transpose(1, 2, 0, 3)
    )

    kT = [np.ascontiguousarray(kbf[b].T) for b in range(B)]
    kn = [
        np.ascontiguousarray(kbf[b].reshape(NKT, P, HIDDEN).transpose(1, 0, 2))
        for b in range(B)
    ]

    in_maps = []
    for core in range(8):
        b, h = divmod(core, 2)
        in_maps.append({
            "qT": np.ascontiguousarray(qbf[b, h * SQ:(h + 1) * SQ, :].T),
            "kT": kT[b],
            "kn": kn[b],
            "wqs": wqs,
            "wks": wks,
            "wqs_b": qb,
            "wks_b": kb,
            "w2": w2r,
        })

    res = run_bass_kernel_spmd(
        nc, in_maps, core_ids=list(range(8)), trace=TRACE, **TRACE_KWARGS
    )
    LAST_RESULTS = res

    # Constant output-bias row: score rows sum to 1, so the wvs_b and wo_b
    # contributions are wvs_b @ wo_w + wo_b for every output row.
    bias_row = (wvs_b @ wo_w + wo_b).astype(np.float32)

    out = np.empty((B, S, HIDDEN), np.float32)
    for core in range(8):
        b, h = divmod(core, 2)
        out[b, h * SQ:(h + 1) * SQ, :] = (
            res.results[core]["out"].astype(np.float32) + bias_row
        )
    return out


# revision 15
# speedup vs baseline: 1.1813x; 1.1813x over previous
"""Trainium2 Bass kernel for low-rank-QK multi-head attention.

Reference computation (B=4, S=2048, HIDDEN=2048, HEADS=16, R=128):
    Q = q @ wqs_w + wqs_b                    # [B, S, 16]
    K = k @ wks_w + wks_b                    # [B, S, 16]
    V = k @ wvs_w + wvs_b                    # [B, S, 2048]   (v input unused)
    logits = Q @ K^T / sqrt(128)             # [B, S, S]
    score = softmax(logits, -1)
    out = (score @ V) @ wo_w + wo_b          # [B, S, 2048]

Sharding: 8 cores = (batch b, query-half h).  Each core handles the full
key set of its batch and a 1024-row query slice.

Algebraic restructuring (host precompute on weights only):
  score @ (k @ wvs) @ wo  =  (score @ k) @ (wvs @ wo)  =  (score @ k) @ W2
W2 = wvs_w @ wo_w is computed on host once per call (weights are runtime
inputs), removing one of the three S*H*H matmuls entirely.  Softmax
normalization is deferred: ET = exp(logits) @ k, out = (ET @ W2) * 1/Z.
Bias fold: softmax rows sum to 1, so wvs_b/wo_b contribute the constant
row wvs_b @ wo_w + wo_b, added on host.

Device-side dataflow (contraction dims on SBUF partitions, no on-device
transposes; matmul operands bf16, PSUM fp32):
  - host supplies qT = q^T (shard), kT = k^T (batch), kn = k in
    [128, tile, hid] layout, W2 in [128, nblk, chunk, 512] layout.
  - QT[h, sq]  = sum_c wqs_c^T qT_c          (16-row result, zero-padded)
  - KT[h, sk]  = sum_c wks_c^T kT_c
  - logitsT_j[k, q] = KT_j^T QT              (keys on partitions)
  - expT_j = exp(logitsT_j / sqrt(R))        (ScalarE, full 1024-wide)
  - Z accumulated j-outer right behind each exp tile (8 psum columns)
  - ET[hid, q] = sum_j kn_j^T expT_j         (= (exp @ k)^T)
  - out[q, n]  = (sum_c ET_c^T W2_c) * (1/Z[q])   per-partition scale
"""

import math
import sys

import numpy as np

if "/opt/trn_rl_repo" not in sys.path:
    sys.path.insert(0, "/opt/trn_rl_repo")

import ml_dtypes

BF = ml_dtypes.bfloat16

HIDDEN = 2048
HEADS = 16
R = 128
B = 4
S = 2048

P = 128
SQ = 1024  # queries per core
SK = 2048  # keys per core (full batch)
HC = HIDDEN // P  # 16 hidden-dim chunks
NKT = SK // P  # 16 key tiles
NQT = SQ // P  # 8 query tiles
N512_Q = SQ // 512  # 2
N512_K = SK // 512  # 4
N512_D = HIDDEN // 512  # 4
ISQRT_R = 1.0 / math.sqrt(R)

# Module-level knobs for test harness (harness itself only calls kernel()).
TRACE = False
TRACE_KWARGS = {}
LAST_RESULTS = None

_PROG = None


def _emit(tc, nc, mybir, ap):
    """Emit the single-core SPMD program body."""
    from contextlib import ExitStack

    f32 = mybir.dt.float32
    bf16 = mybir.dt.bfloat16
    Exp = mybir.ActivationFunctionType.Exp

    with ExitStack() as ctx:
        # ---- long-lived small tiles -------------------------------------
        small = ctx.enter_context(tc.tile_pool(name="small", bufs=1))
        QTs = small.tile([P, SQ], bf16, name="QTs")  # zero-padded 16->128
        KTs = small.tile([P, SK], bf16, name="KTs")
        ones = small.tile([P, 1], bf16, name="ones")
        recip = small.tile([P, NQT], f32, name="recip")
        qb = small.tile([HEADS, 1], f32, name="qb")
        kb = small.tile([HEADS, 1], f32, name="kb")
        wqs_sb = small.tile([P, HC, HEADS], bf16, name="wqs_sb")
        wks_sb = small.tile([P, HC, HEADS], bf16, name="wks_sb")

        # W2 stream pool lives below kn/expT on the stack so its DMAs don't
        # serialize behind the ET phase's last reads of that space (and so
        # the later kn/expT releases keep LIFO order).
        w2s = ctx.enter_context(tc.tile_pool(name="w2s", bufs=2))
        w2_tiles = []

        expT_pool = tc.alloc_tile_pool(name="expT", bufs=1)
        expT = expT_pool.tile([P, NKT, SQ], bf16, name="expT")
        kn_pool = tc.alloc_tile_pool(name="knp", bufs=1)
        kn_sb = kn_pool.tile([P, NKT, HIDDEN], bf16, name="kn_sb")

        def load_w2(n, eng):
            t = w2s.tile([P, HC, 512], bf16, name="w2_t", tag="w2")
            eng.dma_start(t[:], ap["w2"][:, n, :, :])
            w2_tiles.append(t)


        nc.vector.memset(ones[:], 1.0)
        nc.vector.memset(QTs[:], 0.0)
        nc.vector.memset(KTs[:], 0.0)
        nc.sync.dma_start(qb[:], ap["wqs_b"][:])
        nc.sync.dma_start(kb[:], ap["wks_b"][:])
        nc.sync.dma_start(wqs_sb[:], ap["wqs"].rearrange("(c p) h -> p c h", p=P))
        nc.sync.dma_start(wks_sb[:], ap["wks"].rearrange("(c p) h -> p c h", p=P))
        # (kn chunk loads are interleaved into the projection loop below so
        # the latency-critical kT/qT streams win the DMA queues at start.)

        # ================= phase 1: Q/K projections ======================
        with tc.tile_pool(name="kts", bufs=4) as kts, \
             tc.tile_pool(name="qts", bufs=4) as qts, \
             tc.tile_pool(name="ps_kt", bufs=1, space="PSUM") as ps_ktp, \
             tc.tile_pool(name="ps_qt", bufs=1, space="PSUM") as ps_qtp:
            ps_kt = ps_ktp.tile([HEADS, SK], f32, name="ps_kt")
            ps_qt = ps_qtp.tile([HEADS, SQ], f32, name="ps_qt")
            for c in range(HC):
                kt_c = kts.tile([P, SK], bf16, name="kt_c", tag="kt")
                nc.sync.dma_start(kt_c[:], ap["kT"][c * P:(c + 1) * P, :])
                for n in range(N512_K):
                    nc.tensor.matmul(
                        ps_kt[:, n * 512:(n + 1) * 512], wks_sb[:, c, :],
                        kt_c[:, n * 512:(n + 1) * 512],
                        start=(c == 0), stop=(c == HC - 1),
                    )
                qt_c = qts.tile([P, SQ], bf16, name="qt_c", tag="qt")
                nc.sync.dma_start(qt_c[:], ap["qT"][c * P:(c + 1) * P, :])
                for n in range(N512_Q):
                    nc.tensor.matmul(
                        ps_qt[:, n * 512:(n + 1) * 512], wqs_sb[:, c, :],
                        qt_c[:, n * 512:(n + 1) * 512],
                        start=(c == 0), stop=(c == HC - 1),
                    )
            nc.vector.tensor_scalar_add(KTs[0:HEADS, :], ps_kt[:], kb[:])
            nc.vector.tensor_scalar_add(QTs[0:HEADS, :], ps_qt[:], qb[:])

        # Scheduler fence: keep the kn/W2 dma_starts behind the 32 kT/qT
        # entries in the SP queues so the latency-critical proj stream owns
        # the wire first (queue FIFOs drain roughly in step; without the
        # fence the scheduler hoists these and they steal ~40% of the head
        # bandwidth).
        tc.no_sync_barrier()
        for t in range(4):
            nc.sync.dma_start(
                kn_sb[:, 4 * t:4 * (t + 1), :],
                ap["kn"][:, 4 * t:4 * (t + 1), :],
            )
        load_w2(0, nc.sync)
        load_w2(1, nc.sync)

        # ET lives on the right-side stack: its lifetime overlaps (not
        # nests with) the expT/kn pools released before the out phase.
        ET_pool = tc.alloc_tile_pool(name="ETp", bufs=1, side="right")
        ET = ET_pool.tile([P, HC, SQ], bf16, name="ET")

        # ============ phase 2: logitsT + exp; phase 3: Z =================
        with tc.tile_pool(name="ps_l", bufs=2, space="PSUM") as ps_l, \
             tc.tile_pool(name="ps_z", bufs=1, space="PSUM") as ps_zp:
            pz = ps_zp.tile([P, NQT], f32, name="pz")
            for j in range(NKT):
                pl = ps_l.tile([P, SQ], f32, name="ps_l", tag="lT")
                for n in range(N512_Q):
                    nc.tensor.matmul(
                        pl[:, n * 512:(n + 1) * 512],
                        KTs[:, j * P:(j + 1) * P],
                        QTs[:, n * 512:(n + 1) * 512],
                        start=True, stop=True,
                    )
                nc.scalar.activation(expT[:, j, :], pl[:], Exp, scale=ISQRT_R)

            # Z trails each exp tile (j-outer) so the PE absorbs the
            # ScalarE drain instead of idling before phase 4.  All eight
            # pz columns share one PSUM bank whose zero-region is marked
            # by a single start; the other columns' first writes land on
            # pending-zero bytes and write through (correct under both
            # lazy and eager bank-zeroing).
            for j in range(NKT):
                for i in range(NQT):
                    nc.tensor.matmul(
                        pz[:, i:i + 1], expT[:, j, i * P:(i + 1) * P], ones[:],
                        start=(j == 0 and i == 0), stop=(j == NKT - 1),
                        skip_group_check=True,
                    )
            nc.vector.reciprocal(recip[:], pz[:])

            # ============= phase 4: ET[hid, q] = sum_j kn_j^T expT_j =====
            with tc.tile_pool(name="ps_e", bufs=2, space="PSUM") as ps_e:
                for ht in range(HC):
                    for n in range(N512_Q):
                        pe = ps_e.tile([P, 512], f32, name="ps_e", tag="e")
                        for j in range(NKT):
                            nc.tensor.matmul(
                                pe[:],
                                kn_sb[:, j, ht * P:(ht + 1) * P],
                                expT[:, j, n * 512:(n + 1) * 512],
                                start=(j == 0), stop=(j == NKT - 1),
                            )
                        nc.vector.tensor_copy(
                            ET[:, ht, n * 512:(n + 1) * 512], pe[:]
                        )

        kn_pool.release()
        expT_pool.release()

        # ================= phase 5: out = (ET^T @ W2) * recip ============
        with tc.tile_pool(name="outs", bufs=3) as outs, \
             tc.tile_pool(name="ps_o", bufs=2, space="PSUM") as ps_o:
            for n in range(N512_D):
                w2_n = w2_tiles[n]
                for i in range(NQT):
                    po = ps_o.tile([P, 512], f32, name="ps_o", tag="o")
                    for c in range(HC):
                        nc.tensor.matmul(
                            po[:], ET[:, c, i * P:(i + 1) * P], w2_n[:, c, :],
                            start=(c == 0), stop=(c == HC - 1),
                        )
                    ot = outs.tile([P, 512], bf16, name="ot", tag="ot")
                    nc.vector.tensor_scalar_mul(ot[:], po[:], recip[:, i:i + 1])
                    nc.sync.dma_start(
                        ap["out"][i * P:(i + 1) * P, n * 512:(n + 1) * 512], ot[:]
                    )
                if n + 2 < N512_D:
                    load_w2(n + 2, nc.sync)
        ET_pool.release()


def _build_program():
    import concourse.tile as tile
    from concourse import bacc, mybir

    f32 = mybir.dt.float32
    bf16 = mybir.dt.bfloat16

    nc = bacc.Bacc(
        "TRN2", debug=False, num_devices=8, dynamic_dma_scratch_size=512
    )

    ap = {
        "qT": nc.dram_tensor("qT", (HIDDEN, SQ), bf16, kind="ExternalInput").ap(),
        "kT": nc.dram_tensor("kT", (HIDDEN, SK), bf16, kind="ExternalInput").ap(),
        "kn": nc.dram_tensor("kn", (P, NKT, HIDDEN), bf16, kind="ExternalInput").ap(),
        "wqs": nc.dram_tensor("wqs", (HIDDEN, HEADS), bf16, kind="ExternalInput").ap(),
        "wks": nc.dram_tensor("wks", (HIDDEN, HEADS), bf16, kind="ExternalInput").ap(),
        "wqs_b": nc.dram_tensor("wqs_b", (HEADS, 1), f32, kind="ExternalInput").ap(),
        "wks_b": nc.dram_tensor("wks_b", (HEADS, 1), f32, kind="ExternalInput").ap(),
        "w2": nc.dram_tensor(
            "w2", (P, N512_D, HC, 512), bf16, kind="ExternalInput"
        ).ap(),
        "out": nc.dram_tensor("out", (SQ, HIDDEN), bf16, kind="ExternalOutput").ap(),
    }

    with tile.TileContext(nc) as tc:
        _emit(tc, nc, mybir, ap)

    nc.compile()
    return nc


def _get_program():
    global _PROG
    if _PROG is None:
        _PROG = _build_program()
    return _PROG


def kernel(q, k, v, wqs_w, wqs_b, wks_w, wks_b, wvs_w, wvs_b, wo_w, wo_b):
    global LAST_RESULTS
    from concourse.bass_utils import run_bass_kernel_spmd

    nc = _get_program()

    q = np.asarray(q, dtype=np.float32)
    k = np.asarray(k, dtype=np.float32)
    wqs_w = np.asarray(wqs_w, dtype=np.float32)
    wqs_b = np.asarray(wqs_b, dtype=np.float32)
    wks_w = np.asarray(wks_w, dtype=np.float32)
    wks_b = np.asarray(wks_b, dtype=np.float32)
    wvs_w = np.asarray(wvs_w, dtype=np.float32)
    wvs_b = np.asarray(wvs_b, dtype=np.float32)
    wo_w = np.asarray(wo_w, dtype=np.float32)
    wo_b = np.asarray(wo_b, dtype=np.float32)

    qbf = q.astype(BF)
    kbf = k.astype(BF)
    wqs = np.ascontiguousarray(wqs_w.astype(BF))
    wks = np.ascontiguousarray(wks_w.astype(BF))
    qb = np.ascontiguousarray(wqs_b.reshape(HEADS, 1))
    kb = np.ascontiguousarray(wks_b.reshape(HEADS, 1))

    # Weight-only precompute: W2 = wvs @ wo, in the [p, nblk, chunk, col]
    # layout the out-phase DMA wants (one contiguous 16KB/partition block
    # per 512-wide output column group).
    W2 = wvs_w @ wo_w
    w2r = np.ascontiguousarray(
        W2.astype(BF).reshape(HC, P, N512_D, 512).transpose(1, 2, 0, 3)
    )

    kT = [np.ascontiguousarray(kbf[b].T) for b in range(B)]
    kn = [
        np.ascontiguousarray(kbf[b].reshape(NKT, P, HIDDEN).transpose(1, 0, 2))
        for b in range(B)
    ]

    in_maps = []
    for core in range(8):
        b, h = divmod(core, 2)
        in_maps.append({
            "qT": np.ascontiguousarray(qbf[b, h * SQ:(h + 1) * SQ, :].T),
            "kT": kT[b],
            "kn": kn[b],
            "wqs": wqs,
            "wks": wks,
            "wqs_b": qb,
            "wks_b": kb,
            "w2": w2r,
        })

    res = run_bass_kernel_spmd(
        nc, in_maps, core_ids=list(range(8)), trace=TRACE, **TRACE_KWARGS
    )
    LAST_RESULTS = res

    # Constant output-bias row: score rows sum to 1, so the wvs_b and wo_b
    # contributions are wvs_b @ wo_w + wo_b for every output row.
    bias_row = (wvs_b @ wo_w + wo_b).astype(np.float32)

    out = np.empty((B, S, HIDDEN), np.float32)
    for core in range(8):
        b, h = divmod(core, 2)
        out[b, h * SQ:(h + 1) * SQ, :] = (
            res.results[core]["out"].astype(np.float32) + bias_row
        )
    return out


# revision 22
# speedup vs baseline: 1.1872x; 1.0050x over previous
"""Trainium2 Bass kernel for low-rank-QK multi-head attention.

Reference computation (B=4, S=2048, HIDDEN=2048, HEADS=16, R=128):
    Q = q @ wqs_w + wqs_b                    # [B, S, 16]
    K = k @ wks_w + wks_b                    # [B, S, 16]
    V = k @ wvs_w + wvs_b                    # [B, S, 2048]   (v input unused)
    logits = Q @ K^T / sqrt(128)             # [B, S, S]
    score = softmax(logits, -1)
    out = (score @ V) @ wo_w + wo_b          # [B, S, 2048]

Sharding: 8 cores = (batch b, query-half h).  Each core handles the full
key set of its batch and a 1024-row query slice.

Algebraic restructuring (host precompute on weights only):
  score @ (k @ wvs) @ wo  =  (score @ k) @ (wvs @ wo)  =  (score @ k) @ W2
W2 = wvs_w @ wo_w is computed on host once per call (weights are runtime
inputs), removing one of the three S*H*H matmuls entirely.  Softmax
normalization is deferred: ET = exp(logits) @ k, out = (ET @ W2) * 1/Z.
Bias fold: softmax rows sum to 1, so wvs_b/wo_b contribute the constant
row wvs_b @ wo_w + wo_b, added on host.

Device-side dataflow (contraction dims on SBUF partitions, no on-device
transposes; matmul operands bf16, PSUM fp32):
  - host supplies qT = q^T (shard), kT = k^T (batch), kn = k in
    [128, tile, hid] layout, W2 in [128, nblk, chunk, 512] layout.
  - QT[h, sq]  = sum_c wqs_c^T qT_c          (16-row result, zero-padded)
  - KT[h, sk]  = sum_c wks_c^T kT_c
  - logitsT_j[k, q] = KT_j^T QT              (keys on partitions)
  - expT_j = exp(logitsT_j / sqrt(R))        (ScalarE, full 1024-wide)
  - Z accumulated j-outer right behind each exp tile (8 psum columns)
  - ET[hid, q] = sum_j kn_j^T expT_j         (= (exp @ k)^T)
  - out[q, n]  = (sum_c ET_c^T W2_c) * (1/Z[q])   per-partition scale
"""

import math
import sys

import numpy as np

if "/opt/trn_rl_repo" not in sys.path:
    sys.path.insert(0, "/opt/trn_rl_repo")

import ml_dtypes

BF = ml_dtypes.bfloat16

HIDDEN = 2048
HEADS = 16
R = 128
B = 4
S = 2048

P = 128
SQ = 1024  # queries per core
SK = 2048  # keys per core (full batch)
HC = HIDDEN // P  # 16 hidden-dim chunks
NKT = SK // P  # 16 key tiles
NQT = SQ // P  # 8 query tiles
N512_Q = SQ // 512  # 2
N512_K = SK // 512  # 4
N512_D = HIDDEN // 512  # 4
ISQRT_R = 1.0 / math.sqrt(R)

# Module-level knobs for test harness (harness itself only calls kernel()).
TRACE = False
TRACE_KWARGS = {}
LAST_RESULTS = None

_PROG = None


def _emit(tc, nc, mybir, ap):
    """Emit the single-core SPMD program body."""
    from contextlib import ExitStack

    f32 = mybir.dt.float32
    bf16 = mybir.dt.bfloat16
    Exp = mybir.ActivationFunctionType.Exp

    with ExitStack() as ctx:
        # ---- long-lived small tiles -------------------------------------
        small = ctx.enter_context(tc.tile_pool(name="small", bufs=1))
        QTs = small.tile([P, SQ], bf16, name="QTs")  # zero-padded 16->128
        KTs = small.tile([P, SK], bf16, name="KTs")
        ones = small.tile([P, 1], bf16, name="ones")
        recip = small.tile([P, NQT], f32, name="recip")
        qkb = small.tile([HEADS, 2], f32, name="qkb")
        wqk_sb = small.tile([P, 2, HC, HEADS], bf16, name="wqk_sb")

        # W2 stream pool lives below kn/expT on the stack so its DMAs don't
        # serialize behind the ET phase's last reads of that space (and so
        # the later kn/expT releases keep LIFO order).
        w2s = ctx.enter_context(tc.tile_pool(name="w2s", bufs=2))
        w2_tiles = []

        expT_pool = tc.alloc_tile_pool(name="expT", bufs=1)
        expT = expT_pool.tile([P, NKT, SQ], bf16, name="expT")
        kn_pool = tc.alloc_tile_pool(name="knp", bufs=1)
        kn_sb = kn_pool.tile([P, NKT, HIDDEN], bf16, name="kn_sb")

        def load_w2(n, eng):
            t = w2s.tile([P, HC, 512], bf16, name="w2_t", tag="w2")
            eng.dma_start(t[:], ap["w2"][:, n, :, :])
            w2_tiles.append(t)


        nc.vector.memset(ones[:], 1.0)
        nc.vector.memset(QTs[:], 0.0)
        nc.vector.memset(KTs[:], 0.0)
        nc.sync.dma_start(qkb[:], ap["qkb"][:])
        nc.sync.dma_start(wqk_sb[:], ap["wqk"][:])
        # (kn chunk loads are interleaved into the projection loop below so
        # the latency-critical kT/qT streams win the DMA queues at start.)

        # ================= phase 1: Q/K projections ======================
        with tc.tile_pool(name="kts", bufs=4) as kts, \
             tc.tile_pool(name="qts", bufs=4) as qts, \
             tc.tile_pool(name="ps_kt", bufs=1, space="PSUM") as ps_ktp, \
             tc.tile_pool(name="ps_qt", bufs=1, space="PSUM") as ps_qtp:
            ps_kt = ps_ktp.tile([HEADS, SK], f32, name="ps_kt")
            ps_qt = ps_qtp.tile([HEADS, SQ], f32, name="ps_qt")
            for c in range(HC):
                kt_c = kts.tile([P, SK], bf16, name="kt_c", tag="kt")
                nc.sync.dma_start(kt_c[:], ap["kT"][c * P:(c + 1) * P, :])
                for n in range(N512_K):
                    nc.tensor.matmul(
                        ps_kt[:, n * 512:(n + 1) * 512], wqk_sb[:, 1, c, :],
                        kt_c[:, n * 512:(n + 1) * 512],
                        start=(c == 0), stop=(c == HC - 1),
                    )
                qt_c = qts.tile([P, SQ], bf16, name="qt_c", tag="qt")
                nc.sync.dma_start(qt_c[:], ap["qT"][c * P:(c + 1) * P, :])
                for n in range(N512_Q):
                    nc.tensor.matmul(
                        ps_qt[:, n * 512:(n + 1) * 512], wqk_sb[:, 0, c, :],
                        qt_c[:, n * 512:(n + 1) * 512],
                        start=(c == 0), stop=(c == HC - 1),
                    )
            nc.vector.tensor_scalar_add(KTs[0:HEADS, :], ps_kt[:], qkb[:, 1:2])
            nc.vector.tensor_scalar_add(QTs[0:HEADS, :], ps_qt[:], qkb[:, 0:1])

        # Scheduler fence: keep the kn/W2 dma_starts behind the 32 kT/qT
        # entries in the SP queues so the latency-critical proj stream owns
        # the wire first (queue FIFOs drain roughly in step; without the
        # fence the scheduler hoists these and they steal ~40% of the head
        # bandwidth).
        tc.no_sync_barrier()
        for t in range(4):
            nc.sync.dma_start(
                kn_sb[:, 4 * t:4 * (t + 1), :],
                ap["kn"][:, 4 * t:4 * (t + 1), :],
            )
        load_w2(0, nc.sync)
        load_w2(1, nc.sync)

        # ET lives on the right-side stack: its lifetime overlaps (not
        # nests with) the expT/kn pools released before the out phase.
        ET_pool = tc.alloc_tile_pool(name="ETp", bufs=1, side="right")
        ET = ET_pool.tile([P, HC, SQ], bf16, name="ET")

        # ============ phase 2: logitsT + exp; phase 3: Z =================
        with tc.tile_pool(name="ps_l", bufs=2, space="PSUM") as ps_l, \
             tc.tile_pool(name="ps_z", bufs=1, space="PSUM") as ps_zp:
            pz = ps_zp.tile([P, NQT], f32, name="pz")
            for j in range(NKT):
                pl = ps_l.tile([P, SQ], f32, name="ps_l", tag="lT")
                for n in range(N512_Q):
                    nc.tensor.matmul(
                        pl[:, n * 512:(n + 1) * 512],
                        KTs[:, j * P:(j + 1) * P],
                        QTs[:, n * 512:(n + 1) * 512],
                        start=True, stop=True,
                    )
                nc.scalar.activation(expT[:, j, :], pl[:], Exp, scale=ISQRT_R)

            # Z trails each exp tile (j-outer) so the PE absorbs the
            # ScalarE drain instead of idling before phase 4.  All eight
            # pz columns share one PSUM bank whose zero-region is marked
            # by a single start; the other columns' first writes land on
            # pending-zero bytes and write through (correct under both
            # lazy and eager bank-zeroing).
            for j in range(NKT):
                for i in range(NQT):
                    nc.tensor.matmul(
                        pz[:, i:i + 1], expT[:, j, i * P:(i + 1) * P], ones[:],
                        start=(j == 0 and i == 0), stop=(j == NKT - 1),
                        skip_group_check=True,
                    )
            nc.vector.reciprocal(recip[:], pz[:])

            # ============= phase 4: ET[hid, q] = sum_j kn_j^T expT_j =====
            with tc.tile_pool(name="ps_e", bufs=2, space="PSUM") as ps_e:
                for ht in range(HC):
                    for n in range(N512_Q):
                        pe = ps_e.tile([P, 512], f32, name="ps_e", tag="e")
                        for j in range(NKT):
                            nc.tensor.matmul(
                                pe[:],
                                kn_sb[:, j, ht * P:(ht + 1) * P],
                                expT[:, j, n * 512:(n + 1) * 512],
                                start=(j == 0), stop=(j == NKT - 1),
                            )
                        nc.vector.tensor_copy(
                            ET[:, ht, n * 512:(n + 1) * 512], pe[:]
                        )

        kn_pool.release()
        expT_pool.release()

        # ================= phase 5: out = (ET^T @ W2) * recip ============
        with tc.tile_pool(name="outs", bufs=3) as outs, \
             tc.tile_pool(name="ps_o", bufs=2, space="PSUM") as ps_o:
            for n in range(N512_D):
                w2_n = w2_tiles[n]
                for i in range(NQT):
                    po = ps_o.tile([P, 512], f32, name="ps_o", tag="o")
                    for c in range(HC):
                        nc.tensor.matmul(
                            po[:], ET[:, c, i * P:(i + 1) * P], w2_n[:, c, :],
                            start=(c == 0), stop=(c == HC - 1),
                        )
                    ot = outs.tile([P, 512], bf16, name="ot", tag="ot")
                    nc.vector.tensor_scalar_mul(ot[:], po[:], recip[:, i:i + 1])
                    nc.sync.dma_start(
                        ap["out"][i * P:(i + 1) * P, n * 512:(n + 1) * 512], ot[:]
                    )
                if n + 2 < N512_D:
                    load_w2(n + 2, nc.sync)
        ET_pool.release()


def _build_program():
    import concourse.tile as tile
    from concourse import bacc, mybir

    f32 = mybir.dt.float32
    bf16 = mybir.dt.bfloat16

    nc = bacc.Bacc(
        "TRN2", debug=False, num_devices=8, dynamic_dma_scratch_size=512
    )

    ap = {
        "qT": nc.dram_tensor("qT", (HIDDEN, SQ), bf16, kind="ExternalInput").ap(),
        "kT": nc.dram_tensor("kT", (HIDDEN, SK), bf16, kind="ExternalInput").ap(),
        "kn": nc.dram_tensor("kn", (P, NKT, HIDDEN), bf16, kind="ExternalInput").ap(),
        "wqk": nc.dram_tensor("wqk", (P, 2, HC, HEADS), bf16, kind="ExternalInput").ap(),
        "qkb": nc.dram_tensor("qkb", (HEADS, 2), f32, kind="ExternalInput").ap(),
        "w2": nc.dram_tensor(
            "w2", (P, N512_D, HC, 512), bf16, kind="ExternalInput"
        ).ap(),
        "out": nc.dram_tensor("out", (SQ, HIDDEN), bf16, kind="ExternalOutput").ap(),
    }

    with tile.TileContext(nc) as tc:
        _emit(tc, nc, mybir, ap)

    nc.compile()
    return nc


def _get_program():
    global _PROG
    if _PROG is None:
        _PROG = _build_program()
    return _PROG


def kernel(q, k, v, wqs_w, wqs_b, wks_w, wks_b, wvs_w, wvs_b, wo_w, wo_b):
    global LAST_RESULTS
    from concourse.bass_utils import run_bass_kernel_spmd

    nc = _get_program()

    q = np.asarray(q, dtype=np.float32)
    k = np.asarray(k, dtype=np.float32)
    wqs_w = np.asarray(wqs_w, dtype=np.float32)
    wqs_b = np.asarray(wqs_b, dtype=np.float32)
    wks_w = np.asarray(wks_w, dtype=np.float32)
    wks_b = np.asarray(wks_b, dtype=np.float32)
    wvs_w = np.asarray(wvs_w, dtype=np.float32)
    wvs_b = np.asarray(wvs_b, dtype=np.float32)
    wo_w = np.asarray(wo_w, dtype=np.float32)
    wo_b = np.asarray(wo_b, dtype=np.float32)

    qbf = q.astype(BF)
    kbf = k.astype(BF)
    wqk = np.ascontiguousarray(np.stack(
        [wqs_w.astype(BF).reshape(HC, P, HEADS).transpose(1, 0, 2),
         wks_w.astype(BF).reshape(HC, P, HEADS).transpose(1, 0, 2)], axis=1
    ))
    qkb = np.ascontiguousarray(
        np.stack([wqs_b, wks_b], axis=1).astype(np.float32)
    )

    # Weight-only precompute: W2 = wvs @ wo, in the [p, nblk, chunk, col]
    # layout the out-phase DMA wants (one contiguous 16KB/partition block
    # per 512-wide output column group).
    W2 = wvs_w @ wo_w
    w2r = np.ascontiguousarray(
        W2.astype(BF).reshape(HC, P, N512_D, 512).transpose(1, 2, 0, 3)
    )

    kT = [np.ascontiguousarray(kbf[b].T) for b in range(B)]
    kn = [
        np.ascontiguousarray(kbf[b].reshape(NKT, P, HIDDEN).transpose(1, 0, 2))
        for b in range(B)
    ]

    in_maps = []
    for core in range(8):
        b, h = divmod(core, 2)
        in_maps.append({
            "qT": np.ascontiguousarray(qbf[b, h * SQ:(h + 1) * SQ, :].T),
            "kT": kT[b],
            "kn": kn[b],
            "wqk": wqk,
            "qkb": qkb,
            "w2": w2r,
        })

    res = run_bass_kernel_spmd(
        nc, in_maps, core_ids=list(range(8)), trace=TRACE, **TRACE_KWARGS
    )
    LAST_RESULTS = res

    # Constant output-bias row: score rows sum to 1, so the wvs_b and wo_b
    # contributions are wvs_b @ wo_w + wo_b for every output row.
    bias_row = (wvs_b @ wo_w + wo_b).astype(np.float32)

    out = np.empty((B, S, HIDDEN), np.float32)
    for core in range(8):
        b, h = divmod(core, 2)
        out[b, h * SQ:(h + 1) * SQ, :] = (
            res.results[core]["out"].astype(np.float32) + bias_row
        )
    return out
